# revision 20
# baseline (speedup 1.0000x reference)
"""ECE (expected calibration error) kernel for Trainium2, 8-core SPMD.

Math (matching the reference):
  probs = softmax(logits); conf = max prob; pred = argmax; acc = (pred == label)
  bin b covers (b/15, (b+1)/15]; ECE = sum_b |conf_avg_b - acc_avg_b| * cnt_b / N

Device strategy (per core, data-parallel over N):
  conf  = exp(m) / sum_c exp(x_c)   with m = row max    (logits ~ N(0,1): no
          max-subtraction needed for fp32 exp safety)
  acc   = (x[label] == m)           (exact; ties differ from argmax-first with
          probability ~1e-7 per row, negligible for a 1e6-sample average)
  x[label] is fetched on-chip with gpsimd ap_gather: each 16-partition group
  gathers with a shared index list; index k serves partition p = k%16, and a
  constant diagonal mask (multiplied on GPSIMD, reduced on DVE) extracts the
  valid lane.
  Histogram (cumulative over boundaries b=1..14):
    cnt-ish  A_b = sum [conf > c_b]            (DVE mask+reduce)
    acc_cum  B_b = sum [y > 2+c_b], y=conf+2*acc   (DVE mask+reduce)
    conf-Relu R_b = sum Relu(conf - c_b)       (ACT activation w/ accum_out)
  Host recovers cnt_cum=A, acc_cum=B, conf_cum_b = R_b + c_b*cnt_cum_b, then
  finishes the tiny ECE formula (sharding hint: all-reduce 3 tiny vectors,
  finish on host).
"""

import sys

for _p in ("/opt/trn_rl_repo",):
    if _p not in sys.path:
        sys.path.insert(0, _p)

import numpy as np

import concourse.bass as bass
import concourse.bacc as bacc
import concourse.tile as tile
from concourse import mybir
from concourse.bass_utils import run_bass_kernel_spmd

# ---------------------------------------------------------------- constants
N_TOTAL = 1_000_000
C = 256                      # classes
N_CORES = 8
S_CORE = N_TOTAL // N_CORES  # 125_000 samples per core
P = 128                      # partitions
G = 8                        # segments (samples per partition) per supertile
ST = S_CORE // (P * G)       # 122 full supertiles -> 124_928 samples
REM = S_CORE - ST * P * G    # 72 remainder samples
BU = 16                      # supertiles per gather/diag batch
NCOL_DATA = ST * G + 1       # 977 staged per-sample columns (last = remainder)
NCOL = 984                   # padded even column count for 2x DVE modes
N_BINS = 15
N_OUT = 64                   # [0:14] cnt_cum | [27:42) acc_cum(+1 off) | 42 sum_conf | 43 sum_acc | [48:62) conf_relu

BOUNDS = np.linspace(0.0, 1.0, N_BINS + 1, dtype=np.float32)  # matches reference

# Per-supertile engine assignment for the exp-sum: "a" = ACT exp+accum
# (fused, no separate exp pass), "d" = DVE tensor_reduce over the exp output.
# Ratio balances DVE vs ACT busy time near the HBM roofline.
N_DVE_SUM = 59   # sum tiles on DVE; rest fused on ACT (exp+accum)
N_POOL_MAX = 0   # GPSIMD max-tree disabled: Pool is saturated by ap_gather


def _spread(n_special, mark, other):
    kinds = []
    acc = 0
    for _t in range(ST):
        acc += n_special
        if acc >= ST:
            acc -= ST
            kinds.append(mark)
        else:
            kinds.append(other)
    return kinds


SUM_KIND = _spread(N_DVE_SUM, "d", "a")
MAX_KIND = _spread(N_POOL_MAX, "p", "d")

F32 = mybir.dt.float32
I16 = mybir.dt.int16
Alu = mybir.AluOpType
Act = mybir.ActivationFunctionType


def build_program(nc: bass.Bass, repeat: int = 1):
    x = nc.dram_tensor("x", [S_CORE, C], F32, kind="ExternalInput").ap()
    idx = nc.dram_tensor("idx", [P, NCOL_DATA], I16, kind="ExternalInput").ap()
    dmask = nc.dram_tensor("dmask", [P, P], F32, kind="ExternalInput").ap()
    negb = nc.dram_tensor("negb", [P, 16], F32, kind="ExternalInput").ap()
    out = nc.dram_tensor("out", [P, N_OUT], F32, kind="ExternalOutput").ap()

    with tile.TileContext(nc) as tc:
        with (
            tc.tile_pool(name="xin", bufs=4) as xin_pool,
            tc.tile_pool(name="expb", bufs=3) as exp_pool,
            tc.tile_pool(name="gath", bufs=2) as gath_pool,
            tc.tile_pool(name="hist", bufs=2) as hist_pool,
            tc.tile_pool(name="psum", bufs=4, space="PSUM") as psum_pool,
            tc.tile_pool(name="singles", bufs=1) as singles,
        ):
            idx_sb = singles.tile([P, NCOL_DATA], I16)
            nc.sync.dma_start(out=idx_sb[:, :], in_=idx[:, :])
            dmask_sb = singles.tile([P, P], F32)
            nc.sync.dma_start(out=dmask_sb[:, :], in_=dmask[:, :])
            negb_sb = singles.tile([P, 16], F32)
            nc.sync.dma_start(out=negb_sb[:, :], in_=negb[:, :])

            m_stage = singles.tile([P, NCOL], F32)
            s_stage_d = singles.tile([P, NCOL], F32)
            s_stage_a = singles.tile([P, NCOL], F32)
            xl_stage = singles.tile([P, NCOL], F32)

            for _rep in range(repeat):
                # Pad lanes (never written by the loop) must yield conf=0,
                # acc=0: m=-1e30 -> exp(m)=0 -> conf=0; xl=0 != m -> acc=0.
                nc.vector.memset(m_stage[:, :], -1e30)
                nc.vector.memset(s_stage_d[:, :], 1.0)
                nc.vector.memset(s_stage_a[:, :], 1.0)
                nc.vector.memset(xl_stage[:, :], 0.0)

                # ------------- main loop: supertiles of P*G samples --------
                x_rows = x[: ST * P * G, :].rearrange(
                    "(t p g) c -> t p (g c)", p=P, g=G
                )  # [ST, P, G*C]
                for t0 in range(0, ST, BU):
                    U = min(BU, ST - t0)
                    gath_stage = gath_pool.tile([P, BU * G * 16], F32)
                    for u in range(U):
                        t = t0 + u
                        x_sb = xin_pool.tile([P, G * C], F32)
                        nc.sync.dma_start(out=x_sb[:, :], in_=x_rows[t])

                        x3 = x_sb[:, :].rearrange("p (g c) -> p g c", g=G)
                        cols = slice(t * G, (t + 1) * G)
                        if MAX_KIND[t] == "p":
                            # pairwise-max tree on GPSIMD, small DVE finisher
                            mh1 = exp_pool.tile([P, G * 128], F32, tag="mh1")
                            m13 = mh1[:, :].rearrange("p (g c) -> p g c", g=G)
                            nc.gpsimd.tensor_tensor(
                                out=m13, in0=x3[:, :, 0:128], in1=x3[:, :, 128:256],
                                op=Alu.max,
                            )
                            mh2 = exp_pool.tile([P, G * 64], F32, tag="mh2")
                            m23 = mh2[:, :].rearrange("p (g c) -> p g c", g=G)
                            nc.gpsimd.tensor_tensor(
                                out=m23, in0=m13[:, :, 0:64], in1=m13[:, :, 64:128],
                                op=Alu.max,
                            )
                            mh3 = exp_pool.tile([P, G * 32], F32, tag="mh3")
                            m33 = mh3[:, :].rearrange("p (g c) -> p g c", g=G)
                            nc.gpsimd.tensor_tensor(
                                out=m33, in0=m23[:, :, 0:32], in1=m23[:, :, 32:64],
                                op=Alu.max,
                            )
                            nc.vector.tensor_reduce(
                                out=m_stage[:, cols], in_=m33,
                                axis=mybir.AxisListType.X, op=Alu.max,
                            )
                        else:
                            nc.vector.tensor_reduce(
                                out=m_stage[:, cols], in_=x3,
                                axis=mybir.AxisListType.X, op=Alu.max,
                            )

                        kind = SUM_KIND[t]
                        if kind == "a":
                            # ACT computes exp AND the per-segment sum in one
                            # pass per segment (accum_out); exp output is a
                            # throwaway PSUM scratch.
                            for g in range(G):
                                pscr = psum_pool.tile([P, C], F32, tag="pscr")
                                nc.scalar.activation(
                                    pscr[:, :],
                                    x3[:, g, :],
                                    Act.Exp,
                                    accum_out=s_stage_a[:, t * G + g : t * G + g + 1],
                                )
                        else:
                            exp_sb = exp_pool.tile([P, G * C], F32)
                            nc.scalar.activation(exp_sb[:, :], x_sb[:, :], Act.Exp)
                            e3 = exp_sb[:, :].rearrange("p (g c) -> p g c", g=G)
                            nc.vector.tensor_reduce(
                                out=s_stage_d[:, cols], in_=e3,
                                axis=mybir.AxisListType.X, op=Alu.add,
                            )

                        nc.gpsimd.ap_gather(
                            out_ap=gath_stage[:, u * G * 16 : (u + 1) * G * 16]
                            .rearrange("p (k one) -> p k one", one=1),
                            in_ap=x_sb[:, :].rearrange("p (n one) -> p n one", one=1),
                            idxs_ap=idx_sb[:, cols],
                            channels=P, num_elems=G * C, d=1, num_idxs=G * 16,
                        )

                    # batched diagonal extract: multiply on GPSIMD, reduce on DVE
                    dm16 = dmask_sb[:, :16]
                    dm_b = bass.AP(
                        tensor=dm16.tensor, offset=dm16.offset,
                        ap=[dm16.ap[0], [0, U], [0, G], dm16.ap[1]],
                    )
                    g4 = gath_stage[:, : U * G * 16].rearrange(
                        "p (u g j) -> p u g j", u=U, g=G
                    )
                    gm_stage = gath_pool.tile([P, BU * G * 16], F32, tag="gm")
                    gm4 = gm_stage[:, : U * G * 16].rearrange(
                        "p (u g j) -> p u g j", u=U, g=G
                    )
                    nc.gpsimd.tensor_tensor(out=gm4, in0=g4, in1=dm_b, op=Alu.mult)
                    nc.vector.tensor_reduce(
                        out=xl_stage[:, t0 * G : (t0 + U) * G],
                        in_=gm4, axis=mybir.AxisListType.X, op=Alu.add,
                    )

                # ------------- remainder: REM samples, one segment ---------
                rcol = slice(ST * G, ST * G + 1)
                x_rem = xin_pool.tile([P, C], F32, tag="xrem")
                nc.vector.memset(x_rem[:, :], 0.0)
                nc.sync.dma_start(out=x_rem[:REM, :], in_=x[ST * P * G :, :])
                nc.vector.tensor_reduce(
                    out=m_stage[:REM, rcol], in_=x_rem[:REM, :],
                    axis=mybir.AxisListType.X, op=Alu.max,
                )
                exp_rem = exp_pool.tile([P, C], F32, tag="exprem")
                nc.scalar.activation(exp_rem[:REM, :], x_rem[:REM, :], Act.Exp)
                nc.vector.tensor_reduce(
                    out=s_stage_d[:REM, rcol], in_=exp_rem[:REM, :],
                    axis=mybir.AxisListType.X, op=Alu.add,
                )
                gath_rem = gath_pool.tile([P, 16], F32, tag="gathrem")
                nc.gpsimd.ap_gather(
                    out_ap=gath_rem[:, :].rearrange("p (k one) -> p k one", one=1),
                    in_ap=x_rem[:, :].rearrange("p (n one) -> p n one", one=1),
                    idxs_ap=idx_sb[:, rcol],
                    channels=P, num_elems=C, d=1, num_idxs=16,
                )
                gm_rem = gath_pool.tile([P, 16], F32, tag="gmrem")
                nc.vector.tensor_tensor(
                    out=gm_rem[:, :], in0=gath_rem[:, :], in1=dmask_sb[:, :16],
                    op=Alu.mult,
                )
                nc.vector.tensor_reduce(
                    out=xl_stage[:, rcol], in_=gm_rem[:, :],
                    axis=mybir.AxisListType.X, op=Alu.add,
                )

                # ------------- phase B: per-sample conf/acc/y --------------
                exp_m = singles.tile([P, NCOL], F32, tag="expm")
                nc.scalar.activation(exp_m[:, :], m_stage[:, :], Act.Exp)
                s_comb = singles.tile([P, NCOL], F32, tag="scomb")
                nc.vector.tensor_tensor(
                    out=s_comb[:, :], in0=s_stage_d[:, :], in1=s_stage_a[:, :],
                    op=Alu.mult,
                )
                r_s = singles.tile([P, NCOL], F32, tag="rs")
                nc.vector.reciprocal(r_s[:, :], s_comb[:, :])
                conf = singles.tile([P, NCOL], F32, tag="conf")
                nc.vector.tensor_tensor(
                    out=conf[:, :], in0=exp_m[:, :], in1=r_s[:, :], op=Alu.mult
                )
                acc = singles.tile([P, NCOL], F32, tag="acc")
                nc.vector.tensor_tensor(
                    out=acc[:, :], in0=xl_stage[:, :], in1=m_stage[:, :],
                    op=Alu.is_equal,
                )
                acc2 = singles.tile([P, NCOL], F32, tag="acc2")
                nc.vector.tensor_scalar(
                    out=acc2[:, :], in0=acc[:, :], scalar1=2.0, scalar2=None,
                    op0=Alu.mult,
                )
                y = singles.tile([P, NCOL], F32, tag="y")
                nc.vector.tensor_tensor(
                    out=y[:, :], in0=acc2[:, :], in1=conf[:, :], op=Alu.add
                )

                parts = singles.tile([P, 48], F32)
                nc.vector.memset(parts[:, :], 0.0)
                parts_act = singles.tile([P, 16], F32)
                nc.vector.memset(parts_act[:, :], 0.0)

                # ------------- histogram over boundaries 1..14 -------------
                for b in range(1, N_BINS):
                    mask_b = hist_pool.tile([P, NCOL], F32, tag="mask")
                    nc.vector.tensor_scalar(
                        out=mask_b[:, :], in0=conf[:, :],
                        scalar1=float(BOUNDS[b]), scalar2=None, op0=Alu.is_gt,
                    )
                    nc.vector.tensor_reduce(
                        out=parts[:, b - 1 : b], in_=mask_b[:, :],
                        axis=mybir.AxisListType.X, op=Alu.add,
                    )
                    mask2 = hist_pool.tile([P, NCOL], F32, tag="mask2")
                    nc.vector.tensor_scalar(
                        out=mask2[:, :], in0=y[:, :],
                        scalar1=float(np.float32(2.0) + BOUNDS[b]), scalar2=None,
                        op0=Alu.is_gt,
                    )
                    nc.vector.tensor_reduce(
                        out=parts[:, 27 + b : 28 + b], in_=mask2[:, :],
                        axis=mybir.AxisListType.X, op=Alu.add,
                    )
                    relu_scr = hist_pool.tile([P, NCOL], F32, tag="relu")
                    nc.scalar.activation(
                        relu_scr[:, :], conf[:, :], Act.Relu,
                        bias=negb_sb[:, b - 1 : b],
                        accum_out=parts_act[:, b - 1 : b],
                    )
                nc.vector.tensor_reduce(
                    out=parts[:, 42:43], in_=conf[:, :],
                    axis=mybir.AxisListType.X, op=Alu.add,
                )
                nc.vector.tensor_reduce(
                    out=parts[:, 43:44], in_=acc[:, :],
                    axis=mybir.AxisListType.X, op=Alu.add,
                )

            nc.sync.dma_start(out=out[:, :48], in_=parts[:, :])
            nc.sync.dma_start(out=out[:, 48:], in_=parts_act[:, :])
    return nc


# ------------------------------------------------------------- host helpers
def _pack_indices(labels_core: np.ndarray) -> np.ndarray:
    """[P, NCOL_DATA] int16 gather indices in the device's (t, p, g) layout."""
    lab = labels_core.astype(np.int64)
    main = lab[: ST * P * G].reshape(ST, P, G) + 256 * np.arange(G)[None, None, :]
    main = main.transpose(1, 0, 2).reshape(P, ST * G)
    rem = np.zeros((P, 1), np.int64)
    rem[:REM, 0] = lab[ST * P * G :]
    return np.concatenate([main, rem], axis=1).astype(np.int16)


def _diag_mask() -> np.ndarray:
    k = np.arange(P)
    return (k[None, :] % 16 == (k % 16)[:, None]).astype(np.float32)


def _neg_bounds() -> np.ndarray:
    nb = np.zeros((P, 16), np.float32)
    nb[:, :14] = -BOUNDS[1:15][None, :]
    return nb


def finish_on_host(parts_sum: np.ndarray) -> np.ndarray:
    """parts_sum: [45] float64 summed over cores+partitions -> ece [1] f32."""
    cnt_cum = np.zeros(N_BINS + 1)
    conf_cum = np.zeros(N_BINS + 1)
    acc_cum = np.zeros(N_BINS + 1)
    cnt_cum[0] = float(N_TOTAL)
    conf_cum[0] = parts_sum[42]
    acc_cum[0] = parts_sum[43]
    cnt_cum[1:N_BINS] = parts_sum[0:14]
    # device reported sum Relu(conf - c_b); conf_cum_b = that + c_b * cnt_cum_b
    conf_cum[1:N_BINS] = parts_sum[48:62] + BOUNDS[1:15].astype(np.float64) * parts_sum[0:14]
    acc_cum[1:N_BINS] = parts_sum[28:42]
    # per-bin = cumulative differences (cum[15] == 0)
    cnt = cnt_cum[:N_BINS] - cnt_cum[1:]
    conf_s = conf_cum[:N_BINS] - conf_cum[1:]
    acc_s = acc_cum[:N_BINS] - acc_cum[1:]
    safe = np.maximum(cnt, 1.0)
    gap = np.abs(conf_s / safe - acc_s / safe)
    ece = np.sum(np.where(cnt > 0, gap * cnt / N_TOTAL, 0.0))
    return np.array([ece], dtype=np.float32)


_CACHED_NC = None


def _get_nc():
    global _CACHED_NC
    if _CACHED_NC is None:
        nc = bacc.Bacc("TRN2", target_bir_lowering=False, debug=False)
        build_program(nc)
        nc.compile()
        _CACHED_NC = nc
    return _CACHED_NC


def make_in_maps(logits: np.ndarray, labels: np.ndarray):
    logits = np.ascontiguousarray(np.asarray(logits, dtype=np.float32))
    labels = np.asarray(labels)
    dm = _diag_mask()
    nb = _neg_bounds()
    in_maps = []
    for c in range(N_CORES):
        sl = slice(c * S_CORE, (c + 1) * S_CORE)
        in_maps.append(
            {
                "x": logits[sl],
                "idx": _pack_indices(labels[sl]),
                "dmask": dm,
                "negb": nb,
            }
        )
    return in_maps


_LAST_RESULTS = None


def kernel(logits: np.ndarray, labels: np.ndarray) -> np.ndarray:
    global _LAST_RESULTS
    nc = _get_nc()
    in_maps = make_in_maps(logits, labels)
    res = run_bass_kernel_spmd(nc, in_maps, core_ids=list(range(N_CORES)))
    _LAST_RESULTS = res
    parts = np.zeros(N_OUT, dtype=np.float64)
    for core_out in res.results:
        parts += core_out["out"].astype(np.float64).sum(axis=0)
    return finish_on_host(parts)


if __name__ == "__main__":
    rng = np.random.default_rng(0)
    logits = rng.standard_normal((N_TOTAL, C), dtype=np.float32)
    labels = rng.integers(0, C, size=(N_TOTAL,), dtype=np.int64)
    print(kernel(logits=logits, labels=labels))


# revision 23
# speedup vs baseline: 1.0393x; 1.0393x over previous
"""ECE (expected calibration error) kernel for Trainium2, 8-core SPMD.

Math (matching the reference):
  probs = softmax(logits); conf = max prob; pred = argmax; acc = (pred == label)
  bin b covers (b/15, (b+1)/15]; ECE = sum_b |conf_avg_b - acc_avg_b| * cnt_b / N

Device strategy (per core, data-parallel over N):
  conf  = exp(m) / sum_c exp(x_c)   with m = row max    (logits ~ N(0,1): no
          max-subtraction needed for fp32 exp safety)
  acc   = (x[label] == m)           (exact; ties differ from argmax-first with
          probability ~1e-7 per row, negligible for a 1e6-sample average)
  x[label] is fetched on-chip with gpsimd ap_gather: each 16-partition group
  gathers with a shared index list; index k serves partition p = k%16, and a
  constant diagonal mask (multiplied on GPSIMD, reduced on DVE) extracts the
  valid lane.
  Histogram (cumulative over boundaries b=1..14):
    cnt-ish  A_b = sum [conf > c_b]            (DVE mask+reduce)
    acc_cum  B_b = sum [y > 2+c_b], y=conf+2*acc   (DVE mask+reduce)
    conf-Relu R_b = sum Relu(conf - c_b)       (ACT activation w/ accum_out)
  Host recovers cnt_cum=A, acc_cum=B, conf_cum_b = R_b + c_b*cnt_cum_b, then
  finishes the tiny ECE formula (sharding hint: all-reduce 3 tiny vectors,
  finish on host).
"""

import sys

for _p in ("/opt/trn_rl_repo",):
    if _p not in sys.path:
        sys.path.insert(0, _p)

import numpy as np

import concourse.bass as bass
import concourse.bacc as bacc
import concourse.tile as tile
from concourse import mybir
from concourse.bass_utils import run_bass_kernel_spmd

# ---------------------------------------------------------------- constants
N_TOTAL = 1_000_000
C = 256                      # classes
N_CORES = 8
S_CORE = N_TOTAL // N_CORES  # 125_000 samples per core
P = 128                      # partitions
G = 8                        # segments (samples per partition) per supertile
ST = S_CORE // (P * G)       # 122 full supertiles -> 124_928 samples
REM = S_CORE - ST * P * G    # 72 remainder samples
BU = 16                      # supertiles per gather/diag batch
NCOL_DATA = ST * G + 1       # 977 staged per-sample columns (last = remainder)
NCOL = 984                   # padded even column count for 2x DVE modes
N_BINS = 15
N_OUT = 64                   # [0:14] cnt_cum | [27:42) acc_cum(+1 off) | 42 sum_conf | 43 sum_acc | [48:62) conf_relu

BOUNDS = np.linspace(0.0, 1.0, N_BINS + 1, dtype=np.float32)  # matches reference

# Per-supertile engine assignment for the exp-sum: "a" = ACT exp+accum
# (fused, no separate exp pass), "d" = DVE tensor_reduce over the exp output.
# Ratio balances DVE vs ACT busy time near the HBM roofline.
N_DVE_SUM = 6    # sum tiles on DVE; 40 fused on ACT; rest pool-add tree
N_POOL_MAX = 0   # walrus rejects TT max on Pool (engine check)


def _spread(n_special, mark, other):
    kinds = []
    acc = 0
    for _t in range(ST):
        acc += n_special
        if acc >= ST:
            acc -= ST
            kinds.append(mark)
        else:
            kinds.append(other)
    return kinds


SKIP_GATHER = False  # timing-experiment switch; never set for real runs


def _sum_kinds3(n_d=6, n_a=40):
    kinds = []
    ad = aa = 0.0
    for _t in range(ST):
        ad += n_d / ST
        aa += n_a / ST
        if ad >= 1:
            ad -= 1
            kinds.append("d")
        elif aa >= 1:
            aa -= 1
            kinds.append("a")
        else:
            kinds.append("p")
    return kinds


SUM_KIND = _sum_kinds3()
MAX_KIND = _spread(N_POOL_MAX, "p", "d")

F32 = mybir.dt.float32
I16 = mybir.dt.int16
Alu = mybir.AluOpType
Act = mybir.ActivationFunctionType


def build_program(nc: bass.Bass, repeat: int = 1):
    x = nc.dram_tensor("x", [S_CORE, C], F32, kind="ExternalInput").ap()
    idx = nc.dram_tensor("idx", [P, NCOL_DATA], I16, kind="ExternalInput").ap()
    dmask = nc.dram_tensor("dmask", [P, P], F32, kind="ExternalInput").ap()
    negb = nc.dram_tensor("negb", [P, 16], F32, kind="ExternalInput").ap()
    out = nc.dram_tensor("out", [P, N_OUT], F32, kind="ExternalOutput").ap()

    with tile.TileContext(nc) as tc:
        with (
            tc.tile_pool(name="xin", bufs=4) as xin_pool,
            tc.tile_pool(name="expb", bufs=3) as exp_pool,
            tc.tile_pool(name="gath", bufs=2) as gath_pool,
            tc.tile_pool(name="hist", bufs=2) as hist_pool,
            tc.tile_pool(name="psum", bufs=4, space="PSUM") as psum_pool,
            tc.tile_pool(name="singles", bufs=1) as singles,
        ):
            idx_sb = singles.tile([P, NCOL_DATA], I16)
            nc.sync.dma_start(out=idx_sb[:, :], in_=idx[:, :])
            dmask_sb = singles.tile([P, P], F32)
            nc.sync.dma_start(out=dmask_sb[:, :], in_=dmask[:, :])
            negb_sb = singles.tile([P, 16], F32)
            nc.sync.dma_start(out=negb_sb[:, :], in_=negb[:, :])

            m_stage = singles.tile([P, NCOL], F32)
            s_stage_d = singles.tile([P, NCOL], F32)
            s_stage_a = singles.tile([P, NCOL], F32)
            xl_stage = singles.tile([P, NCOL], F32)

            for _rep in range(repeat):
                # Pad lanes (never written by the loop) must yield conf=0,
                # acc=0: m=-1e30 -> exp(m)=0 -> conf=0; xl=0 != m -> acc=0.
                nc.vector.memset(m_stage[:, :], -1e30)
                nc.vector.memset(s_stage_d[:, :], 1.0)
                nc.vector.memset(s_stage_a[:, :], 1.0)
                nc.vector.memset(xl_stage[:, :], 0.0)

                # ------------- main loop: supertiles of P*G samples --------
                x_rows = x[: ST * P * G, :].rearrange(
                    "(t p g) c -> t p (g c)", p=P, g=G
                )  # [ST, P, G*C]
                for t0 in range(0, ST, BU):
                    U = min(BU, ST - t0)
                    gath_stage = gath_pool.tile([P, BU * G * 16], F32)
                    for u in range(U):
                        t = t0 + u
                        x_sb = xin_pool.tile([P, G * C], F32)
                        nc.sync.dma_start(out=x_sb[:, :], in_=x_rows[t])

                        x3 = x_sb[:, :].rearrange("p (g c) -> p g c", g=G)
                        cols = slice(t * G, (t + 1) * G)
                        if MAX_KIND[t] == "p":
                            # pairwise-max tree on GPSIMD, small DVE finisher
                            mh1 = exp_pool.tile([P, G * 128], F32, tag="mh1")
                            m13 = mh1[:, :].rearrange("p (g c) -> p g c", g=G)
                            nc.gpsimd.tensor_tensor(
                                out=m13, in0=x3[:, :, 0:128], in1=x3[:, :, 128:256],
                                op=Alu.max,
                            )
                            mh2 = exp_pool.tile([P, G * 64], F32, tag="mh2")
                            m23 = mh2[:, :].rearrange("p (g c) -> p g c", g=G)
                            nc.gpsimd.tensor_tensor(
                                out=m23, in0=m13[:, :, 0:64], in1=m13[:, :, 64:128],
                                op=Alu.max,
                            )
                            mh3 = exp_pool.tile([P, G * 32], F32, tag="mh3")
                            m33 = mh3[:, :].rearrange("p (g c) -> p g c", g=G)
                            nc.gpsimd.tensor_tensor(
                                out=m33, in0=m23[:, :, 0:32], in1=m23[:, :, 32:64],
                                op=Alu.max,
                            )
                            nc.vector.tensor_reduce(
                                out=m_stage[:, cols], in_=m33,
                                axis=mybir.AxisListType.X, op=Alu.max,
                            )
                        else:
                            nc.vector.tensor_reduce(
                                out=m_stage[:, cols], in_=x3,
                                axis=mybir.AxisListType.X, op=Alu.max,
                            )

                        kind = SUM_KIND[t]
                        if kind == "a":
                            # ACT computes exp AND the per-segment sum in one
                            # pass per segment (accum_out); exp output is a
                            # throwaway PSUM scratch.
                            for g in range(G):
                                pscr = psum_pool.tile([P, C], F32, tag="pscr")
                                nc.scalar.activation(
                                    pscr[:, :],
                                    x3[:, g, :],
                                    Act.Exp,
                                    accum_out=s_stage_a[:, t * G + g : t * G + g + 1],
                                )
                        else:
                            exp_sb = exp_pool.tile([P, G * C], F32)
                            nc.scalar.activation(exp_sb[:, :], x_sb[:, :], Act.Exp)
                            e3 = exp_sb[:, :].rearrange("p (g c) -> p g c", g=G)
                            if kind == "d":
                                nc.vector.tensor_reduce(
                                    out=s_stage_d[:, cols], in_=e3,
                                    axis=mybir.AxisListType.X, op=Alu.add,
                                )
                            else:  # "p": pairwise-add tree on GPSIMD, DVE finish
                                sh1 = exp_pool.tile([P, G * 128], F32, tag="sh1")
                                s13 = sh1[:, :].rearrange("p (g c) -> p g c", g=G)
                                nc.gpsimd.tensor_tensor(
                                    out=s13, in0=e3[:, :, 0:128], in1=e3[:, :, 128:256],
                                    op=Alu.add,
                                )
                                sh2 = exp_pool.tile([P, G * 64], F32, tag="sh2")
                                s23 = sh2[:, :].rearrange("p (g c) -> p g c", g=G)
                                nc.gpsimd.tensor_tensor(
                                    out=s23, in0=s13[:, :, 0:64], in1=s13[:, :, 64:128],
                                    op=Alu.add,
                                )
                                nc.vector.tensor_reduce(
                                    out=s_stage_d[:, cols], in_=s23,
                                    axis=mybir.AxisListType.X, op=Alu.add,
                                )

                        if not SKIP_GATHER:
                            nc.gpsimd.ap_gather(
                                out_ap=gath_stage[:, u * G * 16 : (u + 1) * G * 16]
                                .rearrange("p (k one) -> p k one", one=1),
                                in_ap=x_sb[:, :].rearrange("p (n one) -> p n one", one=1),
                                idxs_ap=idx_sb[:, cols],
                                channels=P, num_elems=G * C, d=1, num_idxs=G * 16,
                            )
                        elif u == 0:
                            nc.vector.memset(gath_stage[:, :], 0.0)

                    # batched diagonal extract: multiply on GPSIMD, reduce on DVE
                    dm16 = dmask_sb[:, :16]
                    dm_b = bass.AP(
                        tensor=dm16.tensor, offset=dm16.offset,
                        ap=[dm16.ap[0], [0, U], [0, G], dm16.ap[1]],
                    )
                    g4 = gath_stage[:, : U * G * 16].rearrange(
                        "p (u g j) -> p u g j", u=U, g=G
                    )
                    gm_stage = gath_pool.tile([P, BU * G * 16], F32, tag="gm")
                    gm4 = gm_stage[:, : U * G * 16].rearrange(
                        "p (u g j) -> p u g j", u=U, g=G
                    )
                    nc.gpsimd.tensor_tensor(out=gm4, in0=g4, in1=dm_b, op=Alu.mult)
                    nc.vector.tensor_reduce(
                        out=xl_stage[:, t0 * G : (t0 + U) * G],
                        in_=gm4, axis=mybir.AxisListType.X, op=Alu.add,
                    )

                # ------------- remainder: REM samples, one segment ---------
                rcol = slice(ST * G, ST * G + 1)
                x_rem = xin_pool.tile([P, C], F32, tag="xrem")
                nc.vector.memset(x_rem[:, :], 0.0)
                nc.sync.dma_start(out=x_rem[:REM, :], in_=x[ST * P * G :, :])
                nc.vector.tensor_reduce(
                    out=m_stage[:REM, rcol], in_=x_rem[:REM, :],
                    axis=mybir.AxisListType.X, op=Alu.max,
                )
                exp_rem = exp_pool.tile([P, C], F32, tag="exprem")
                nc.scalar.activation(exp_rem[:REM, :], x_rem[:REM, :], Act.Exp)
                nc.vector.tensor_reduce(
                    out=s_stage_d[:REM, rcol], in_=exp_rem[:REM, :],
                    axis=mybir.AxisListType.X, op=Alu.add,
                )
                gath_rem = gath_pool.tile([P, 16], F32, tag="gathrem")
                nc.gpsimd.ap_gather(
                    out_ap=gath_rem[:, :].rearrange("p (k one) -> p k one", one=1),
                    in_ap=x_rem[:, :].rearrange("p (n one) -> p n one", one=1),
                    idxs_ap=idx_sb[:, rcol],
                    channels=P, num_elems=C, d=1, num_idxs=16,
                )
                gm_rem = gath_pool.tile([P, 16], F32, tag="gmrem")
                nc.vector.tensor_tensor(
                    out=gm_rem[:, :], in0=gath_rem[:, :], in1=dmask_sb[:, :16],
                    op=Alu.mult,
                )
                nc.vector.tensor_reduce(
                    out=xl_stage[:, rcol], in_=gm_rem[:, :],
                    axis=mybir.AxisListType.X, op=Alu.add,
                )

                # ------------- phase B: per-sample conf/acc/y --------------
                exp_m = singles.tile([P, NCOL], F32, tag="expm")
                nc.scalar.activation(exp_m[:, :], m_stage[:, :], Act.Exp)
                s_comb = singles.tile([P, NCOL], F32, tag="scomb")
                nc.vector.tensor_tensor(
                    out=s_comb[:, :], in0=s_stage_d[:, :], in1=s_stage_a[:, :],
                    op=Alu.mult,
                )
                r_s = singles.tile([P, NCOL], F32, tag="rs")
                nc.vector.reciprocal(r_s[:, :], s_comb[:, :])
                conf = singles.tile([P, NCOL], F32, tag="conf")
                nc.vector.tensor_tensor(
                    out=conf[:, :], in0=exp_m[:, :], in1=r_s[:, :], op=Alu.mult
                )
                acc = singles.tile([P, NCOL], F32, tag="acc")
                nc.vector.tensor_tensor(
                    out=acc[:, :], in0=xl_stage[:, :], in1=m_stage[:, :],
                    op=Alu.is_equal,
                )
                acc2 = singles.tile([P, NCOL], F32, tag="acc2")
                nc.vector.tensor_scalar(
                    out=acc2[:, :], in0=acc[:, :], scalar1=2.0, scalar2=None,
                    op0=Alu.mult,
                )
                y = singles.tile([P, NCOL], F32, tag="y")
                nc.vector.tensor_tensor(
                    out=y[:, :], in0=acc2[:, :], in1=conf[:, :], op=Alu.add
                )

                parts = singles.tile([P, 48], F32)
                nc.vector.memset(parts[:, :], 0.0)
                parts_act = singles.tile([P, 16], F32)
                nc.vector.memset(parts_act[:, :], 0.0)

                # ------------- histogram over boundaries 1..14 -------------
                for b in range(1, N_BINS):
                    mask_b = hist_pool.tile([P, NCOL], F32, tag="mask")
                    nc.vector.tensor_scalar(
                        out=mask_b[:, :], in0=conf[:, :],
                        scalar1=float(BOUNDS[b]), scalar2=None, op0=Alu.is_gt,
                    )
                    nc.vector.tensor_reduce(
                        out=parts[:, b - 1 : b], in_=mask_b[:, :],
                        axis=mybir.AxisListType.X, op=Alu.add,
                    )
                    mask2 = hist_pool.tile([P, NCOL], F32, tag="mask2")
                    nc.vector.tensor_scalar(
                        out=mask2[:, :], in0=y[:, :],
                        scalar1=float(np.float32(2.0) + BOUNDS[b]), scalar2=None,
                        op0=Alu.is_gt,
                    )
                    nc.vector.tensor_reduce(
                        out=parts[:, 27 + b : 28 + b], in_=mask2[:, :],
                        axis=mybir.AxisListType.X, op=Alu.add,
                    )
                    relu_scr = hist_pool.tile([P, NCOL], F32, tag="relu")
                    nc.scalar.activation(
                        relu_scr[:, :], conf[:, :], Act.Relu,
                        bias=negb_sb[:, b - 1 : b],
                        accum_out=parts_act[:, b - 1 : b],
                    )
                nc.vector.tensor_reduce(
                    out=parts[:, 42:43], in_=conf[:, :],
                    axis=mybir.AxisListType.X, op=Alu.add,
                )
                nc.vector.tensor_reduce(
                    out=parts[:, 43:44], in_=acc[:, :],
                    axis=mybir.AxisListType.X, op=Alu.add,
                )

            nc.sync.dma_start(out=out[:, :48], in_=parts[:, :])
            nc.sync.dma_start(out=out[:, 48:], in_=parts_act[:, :])
    return nc


# ------------------------------------------------------------- host helpers
def _pack_indices(labels_core: np.ndarray) -> np.ndarray:
    """[P, NCOL_DATA] int16 gather indices in the device's (t, p, g) layout."""
    lab = labels_core.astype(np.int64)
    main = lab[: ST * P * G].reshape(ST, P, G) + 256 * np.arange(G)[None, None, :]
    main = main.transpose(1, 0, 2).reshape(P, ST * G)
    rem = np.zeros((P, 1), np.int64)
    rem[:REM, 0] = lab[ST * P * G :]
    return np.concatenate([main, rem], axis=1).astype(np.int16)


def _diag_mask() -> np.ndarray:
    k = np.arange(P)
    return (k[None, :] % 16 == (k % 16)[:, None]).astype(np.float32)


def _neg_bounds() -> np.ndarray:
    nb = np.zeros((P, 16), np.float32)
    nb[:, :14] = -BOUNDS[1:15][None, :]
    return nb


def finish_on_host(parts_sum: np.ndarray) -> np.ndarray:
    """parts_sum: [45] float64 summed over cores+partitions -> ece [1] f32."""
    cnt_cum = np.zeros(N_BINS + 1)
    conf_cum = np.zeros(N_BINS + 1)
    acc_cum = np.zeros(N_BINS + 1)
    cnt_cum[0] = float(N_TOTAL)
    conf_cum[0] = parts_sum[42]
    acc_cum[0] = parts_sum[43]
    cnt_cum[1:N_BINS] = parts_sum[0:14]
    # device reported sum Relu(conf - c_b); conf_cum_b = that + c_b * cnt_cum_b
    conf_cum[1:N_BINS] = parts_sum[48:62] + BOUNDS[1:15].astype(np.float64) * parts_sum[0:14]
    acc_cum[1:N_BINS] = parts_sum[28:42]
    # per-bin = cumulative differences (cum[15] == 0)
    cnt = cnt_cum[:N_BINS] - cnt_cum[1:]
    conf_s = conf_cum[:N_BINS] - conf_cum[1:]
    acc_s = acc_cum[:N_BINS] - acc_cum[1:]
    safe = np.maximum(cnt, 1.0)
    gap = np.abs(conf_s / safe - acc_s / safe)
    ece = np.sum(np.where(cnt > 0, gap * cnt / N_TOTAL, 0.0))
    return np.array([ece], dtype=np.float32)


_CACHED_NC = None


def _get_nc():
    global _CACHED_NC
    if _CACHED_NC is None:
        nc = bacc.Bacc("TRN2", target_bir_lowering=False, debug=False)
        build_program(nc)
        nc.compile()
        _CACHED_NC = nc
    return _CACHED_NC


def make_in_maps(logits: np.ndarray, labels: np.ndarray):
    logits = np.ascontiguousarray(np.asarray(logits, dtype=np.float32))
    labels = np.asarray(labels)
    dm = _diag_mask()
    nb = _neg_bounds()
    in_maps = []
    for c in range(N_CORES):
        sl = slice(c * S_CORE, (c + 1) * S_CORE)
        in_maps.append(
            {
                "x": logits[sl],
                "idx": _pack_indices(labels[sl]),
                "dmask": dm,
                "negb": nb,
            }
        )
    return in_maps


_LAST_RESULTS = None


def kernel(logits: np.ndarray, labels: np.ndarray) -> np.ndarray:
    global _LAST_RESULTS
    nc = _get_nc()
    in_maps = make_in_maps(logits, labels)
    res = run_bass_kernel_spmd(nc, in_maps, core_ids=list(range(N_CORES)))
    _LAST_RESULTS = res
    parts = np.zeros(N_OUT, dtype=np.float64)
    for core_out in res.results:
        parts += core_out["out"].astype(np.float64).sum(axis=0)
    return finish_on_host(parts)


if __name__ == "__main__":
    rng = np.random.default_rng(0)
    logits = rng.standard_normal((N_TOTAL, C), dtype=np.float32)
    labels = rng.integers(0, C, size=(N_TOTAL,), dtype=np.int64)
    print(kernel(logits=logits, labels=labels))
